# revision 2
# baseline (speedup 1.0000x reference)
"""ArcFace-style loss kernel for Trainium2 (8 NeuronCores).

Strategy
--------
The heavy tensor is ``weight`` [200000, 192] (153.6 MB f32).  Per (b, m)
embedding row the loss needs ``sum_full[b,m] = sum_c exp(30*cos[b,m,c] - 30)``
plus the cosine at the 4 label columns (tiny, done exactly on host).

Device (per core, classes sharded 8-way -> 25000 classes/core):
  * W^T streamed in fp8 (e4m3), both 96-row K-halves packed per block so each
    block is ONE contiguous DMA.  DoubleRow fp8 matmuls (K=192 as 2x96, one
    matmul per 512-col tile) write psum = AB*cos with AB = 30*1024/ln2 --
    the exponent scale of the fp16 Schraudolph trick below.
  * The exp+reduce work is split across two engines with independent PSUM
    rings (2 slots x 1024 cols each, so neither engine's slot refill waits on
    the other engine):
      - ACT share (~60%): exact table exp in place on PSUM with the fused
        per-row accumulator (free reduction).
      - DVE share (~40%): Schraudolph exp -- tensor_scalar computes
        i16 = trunc(max(psum, -2048) + 2048.5), whose int16 bit pattern read
        as fp16 is ~exp(30cos - 30)*2^9.01; a second 4x-mode tensor_scalar
        (2-byte, SBUF) reduces two converted tiles per pass via accum_out.
  * Raw per-super partial accumulators [128, 36] are DMA'd out; the host
    sums them (A-partials exact, D-partials scaled by KD / 1.0363 -- the
    Schraudolph linear-interp mean bias is a data-independent constant).

Host: l2-normalize x and W, gather the 128 label rows of W for exact cos_l,
all-reduce the 8 partial sums, then the O(B*M*S) ArcFace + Hungarian + BCE
epilogue in float64.  The Schraudolph path only reaches the loss through
log(sum_exp) of a ~40% share; measured final rel err ~1e-3 vs the f64
reference (gate is 2e-2).
"""

import math
from contextlib import ExitStack

import numpy as np

import concourse.bass as bass
import concourse.tile as tile
from concourse import bacc, mybir
from concourse.bass_utils import run_bass_kernel_spmd

# ---- problem constants (hardcoded per contract) ----
B, M, D, NC = 32, 4, 192, 200000
BM = B * M                       # 128 rows
N_CORES = 8
C_SH = NC // N_CORES             # 25000 classes per core
S_SPK = 4
SCALE = 30.0
MARGIN = 0.5
ETA, XI = 2.5, 5.0
COS_M = math.cos(MARGIN)
SIN_M = math.sin(MARGIN)
TH = math.cos(math.pi - MARGIN)
MM = math.sin(math.pi - MARGIN) * MARGIN
EPS = 1e-6

K0 = 96                          # K-half for DoubleRow (2x96 = 192)
SW = 1024                        # psum slot width (2 banks) per engine ring

# fp16-Schraudolph constants: psum = AB*cos; i16 = trunc(max(psum,-2048)+CADD)
LOG2_FP16 = 1024.0 / math.log(2.0)
AB = SCALE * LOG2_FP16           # 44319.5
PRE = math.sqrt(AB)              # per-operand fp8 prescale (210.5)
CLAMP = -2048.0
CADD = 2048.5
# host scale for D partials: value(i16 as fp16) = e^{30cos} * 2^{(CADD-15360)/1024}
# divided by the Schraudolph mean overestimate (2^f ~ 1+f, f~uniform): 1.0363
KD = math.exp(-SCALE) * 2.0 ** ((15360.0 - CADD) / 1024.0) / 1.03633

# per-core column blocks (one contiguous DMA each) and the engine pattern
BLOCKS = [512, 1024, 1024, 2048, 2048, 4096, 4096, 4096, 4096, 1960]
PATTERN = "AADAD"                # cycled over supers: 60% ACT, 40% DVE
NA, ND = 20, 16                  # partial-accumulator columns per engine

DTYPE = "fp8dr"

LAST_EXEC_NS = None
LAST_RESULTS = None

_CACHE = {}


def _plan():
    plan, k = [], 0
    for b in BLOCKS:
        cur, off = [], 0
        while off < b:
            w = min(SW, b - off)
            cur.append((PATTERN[k % len(PATTERN)], w))
            k += 1
            off += w
        plan.append(cur)
    return plan


def _build(dtype_name="fp8dr"):
    assert dtype_name == "fp8dr"
    f32 = mybir.dt.float32
    fp16 = mybir.dt.float16
    fp8 = mybir.dt.float8e4
    i16 = mybir.dt.int16
    AF = mybir.ActivationFunctionType
    Alu = mybir.AluOpType

    plan = _plan()
    assert sum(w for blk in plan for _, w in blk) == C_SH

    nc = bacc.Bacc(
        "TRN2", target_bir_lowering=False, debug=False, num_devices=N_CORES
    )
    wt = nc.dram_tensor("wt", [K0, 2 * C_SH], fp8, kind="ExternalInput").ap()
    xt = nc.dram_tensor("xt", [K0, 2 * BM], fp8, kind="ExternalInput").ap()
    out = nc.dram_tensor("out", [BM, NA + ND], f32, kind="ExternalOutput").ap()

    with tile.TileContext(nc) as tc, ExitStack() as ctx:
        xp = ctx.enter_context(tc.tile_pool(name="x", bufs=1))
        wp = ctx.enter_context(tc.tile_pool(name="w", bufs=len(BLOCKS)))
        pa = ctx.enter_context(tc.tile_pool(name="pa", bufs=2, space="PSUM"))
        pd = ctx.enter_context(tc.tile_pool(name="pd", bufs=2, space="PSUM"))
        accp = ctx.enter_context(tc.tile_pool(name="acc", bufs=1))

        acc = accp.tile([BM, NA + ND], f32, tag="acc")
        nc.gpsimd.memset(acc[:], 0.0)
        accA = acc[:, 0:NA]
        accD = acc[:, NA : NA + ND]
        bias_t = accp.tile([BM, 1], f32, tag="bias")
        nc.gpsimd.memset(bias_t[:], -SCALE)
        # dummy 1-elem Exp pulls the activation-table load off the critical
        # path (overlaps the first W DMA)
        warm = accp.tile([BM, 1], f32, tag="warm")
        nc.scalar.activation(warm[:], bias_t[:], AF.Exp, bias=bias_t[:], scale=0.0)
        # int16 ring: 4 slots; adjacent slot pairs reduced by one 4x-mode pass
        itr = accp.tile([BM, 4 * SW], i16, tag="itr")

        first = True
        ja, jd, g0, dslot = 0, 0, 0, 0
        pend = []  # pending (ring_lo, width) convert outputs awaiting reduce

        def emit_red(force=False):
            nonlocal jd, pend
            if len(pend) == 2 and pend[0][0] + pend[0][1] == pend[1][0]:
                lo = pend[0][0]
                w_ = pend[0][1] + pend[1][1]
                pend = []
            elif force and pend:
                lo, w_ = pend.pop(0)
            else:
                return
            nc.vector.tensor_scalar(
                itr[:, lo : lo + w_].bitcast(fp16),
                itr[:, lo : lo + w_].bitcast(fp16),
                1.0,
                0.0,
                op0=Alu.mult,
                op1=Alu.add,
                accum_out=accD[:, jd : jd + 1],
            )
            jd += 1

        for blk in plan:
            bw = sum(w for _, w in blk)
            w = wp.tile([K0, 2 * bw], fp8, tag="w", name="wblk")
            nc.sync.dma_start(w[:], wt[:, 2 * g0 : 2 * g0 + 2 * bw])
            if first:
                xtile = xp.tile([K0, 2 * BM], fp8, tag="xt")
                nc.sync.dma_start(xtile[:], xt[:, :])
                x3 = xtile[:].rearrange("p (two m) -> p two m", two=2)
                first = False
            w3 = w[:].rearrange("p (two n) -> p two n", two=2)
            off = 0
            for eng, width in blk:
                pool = pa if eng == "A" else pd
                ps = pool.tile([BM, SW], f32, tag="ps", name="ps")
                for t0 in range(0, width, 512):
                    tw = min(512, width - t0)
                    nc.tensor.matmul(
                        ps[:, t0 : t0 + tw],
                        x3,
                        w3[:, :, off + t0 : off + t0 + tw],
                        start=True,
                        stop=True,
                        perf_mode=mybir.MatmulPerfMode.DoubleRow,
                    )
                if eng == "A":
                    nc.scalar.activation(
                        ps[:, :width],
                        ps[:, :width],
                        AF.Exp,
                        bias=bias_t[:],
                        scale=SCALE / AB,
                        accum_out=accA[:, ja : ja + 1],
                    )
                    ja += 1
                else:
                    lo = dslot * SW
                    nc.vector.tensor_scalar(
                        itr[:, lo : lo + width],
                        ps[:, :width],
                        CLAMP,
                        CADD,
                        op0=Alu.max,
                        op1=Alu.add,
                    )
                    pend.append((lo, width))
                    dslot = (dslot + 1) % 4
                    if dslot % 2 == 0:
                        emit_red()
                off += width
            g0 += bw
        while pend:
            emit_red(force=True)
        assert ja <= NA and jd <= ND, (ja, jd)
        nc.sync.dma_start(out, acc[:])
    nc.compile()
    return nc


def _get_nc(dtype_name):
    if dtype_name not in _CACHE:
        _CACHE[dtype_name] = _build(dtype_name)
    return _CACHE[dtype_name]


def _l2n(x, axis=-1):
    n = np.linalg.norm(x.astype(np.float32), axis=axis, keepdims=True)
    return x / np.maximum(n, 1e-12)


def _pack_core(wT_scaled_f8, g0):
    """Per-block contiguous packing of this core's W^T slice.
    wT_scaled_f8: full [D, NC] fp8 array; g0: first class of the core."""
    parts = []
    g = g0
    for b in BLOCKS:
        parts.append(wT_scaled_f8[0:K0, g : g + b])
        parts.append(wT_scaled_f8[K0:D, g : g + b])
        g += b
    return np.ascontiguousarray(np.concatenate(parts, axis=1))


def _device_sumexp(xn, wn, dtype_name, trace=False):
    """Run the 8-core SPMD kernel. xn: [BM, D] f32 normalized rows;
    wn: [NC, D] f32 normalized rows. Returns sum_full [BM] f64."""
    global LAST_EXEC_NS, LAST_RESULTS
    import ml_dtypes

    f8 = np.dtype(ml_dtypes.float8_e4m3)
    xT = (xn.T * PRE).astype(f8)                              # [D, BM]
    xT_packed = np.ascontiguousarray(
        np.concatenate([xT[0:K0], xT[K0:D]], axis=1)
    )                                                         # [96, 256]
    wT = (wn.T * PRE).astype(f8)                              # [D, NC]
    in_maps = []
    for k in range(N_CORES):
        in_maps.append(
            {"wt": _pack_core(wT, k * C_SH), "xt": xT_packed}
        )
    # NTFF tracing is unavailable under this axon client; force it off
    import os as _os

    _os.environ.setdefault("BASS_NEVER_TRACE", "1")
    nc = _get_nc(dtype_name)
    res = None
    last_err = None
    for attempt in range(3):
        try:
            res = run_bass_kernel_spmd(
                nc, in_maps, core_ids=list(range(N_CORES)), trace=trace
            )
            break
        except Exception as e:  # wedged-device NRT errors recover on retry
            last_err = e
            import time as _time

            _time.sleep(2.0)
    if res is None:
        raise last_err
    LAST_EXEC_NS = res.exec_time_ns
    LAST_RESULTS = res
    total = np.zeros(BM, np.float64)
    for k in range(N_CORES):
        o = res.results[k]["out"].astype(np.float64)          # [BM, NA+ND]
        total += o[:, :NA].sum(axis=1) + KD * o[:, NA:].sum(axis=1)
    return total


def kernel(pred_embs, pred_ps, gt_labels, weight):
    pred_embs = np.asarray(pred_embs, dtype=np.float32)
    pred_ps = np.asarray(pred_ps, dtype=np.float32)
    gt_labels = np.asarray(gt_labels)
    weight = np.asarray(weight, dtype=np.float32)

    # --- host marshalling: l2 normalize both operands (f32, like the ref) ---
    x = pred_embs.reshape(BM, D)
    xn = _l2n(x)                                           # [128, 192]
    wn = _l2n(weight)                                      # [200000, 192]

    # --- device: all-class sum of exp(30*cos - 30), sharded over 8 cores ---
    sum_full = _device_sumexp(xn, wn, DTYPE)               # [128] f64
    sum_full = sum_full.reshape(B, M)

    # --- host: labels, mirroring jax.lax.top_k(gt_labels, S_SPK)[1]
    labels = np.argsort(-gt_labels, axis=1, kind="stable")[:, :S_SPK]

    # --- host: exact cos at label columns (128 rows of W) ---
    xn64 = xn.reshape(B, M, D).astype(np.float64)
    wl = _l2n(weight[labels]).astype(np.float64)           # [B, S, D]
    cos_l = np.einsum("bmd,bsd->bms", xn64, wl)            # [B, M, S]

    sin_l = np.sqrt(np.clip(1.0 - cos_l**2, 0.0, 1.0))
    phi_l = cos_l * COS_M - sin_l * SIN_M
    phi_l = np.where(cos_l > TH, phi_l, cos_l - MM)

    # logsumexp with the label column replaced by phi (shift = SCALE)
    adj = (
        sum_full[:, :, None]
        - np.exp(SCALE * cos_l - SCALE)
        + np.exp(SCALE * phi_l - SCALE)
    )
    lse = SCALE + np.log(adj)                              # [B, M, S]
    ce = lse - SCALE * phi_l
    C = np.swapaxes(ce, 1, 2)                              # [B, S, M]

    # Hungarian on 4x4 via brute force over 24 permutations
    import itertools

    perms = np.array(list(itertools.permutations(range(S_SPK))), np.int64)
    pc = C[:, np.arange(S_SPK)[None, :], perms].sum(-1)    # [B, P]
    best = np.argmin(pc, axis=1)
    col = perms[best]                                      # [B, S]

    matched = C[np.arange(B)[:, None], np.arange(S_SPK)[None, :], col]
    L_spk = matched.mean(axis=1)                           # [B]

    t_exist = np.zeros((B, M), np.float64)
    t_exist[np.arange(B)[:, None], col] = 1.0
    p = np.clip(pred_ps.astype(np.float64), EPS, 1.0 - EPS)
    L_exist = -(t_exist * np.log(p) + (1.0 - t_exist) * np.log(1.0 - p)).mean(axis=1)
    L_stop = -np.log(np.clip(pred_ps[:, -1].astype(np.float64), EPS, 1.0 - EPS))

    L_total = 0.01 * L_spk + ETA * L_exist + XI * L_stop
    return (
        np.float32(L_total.mean()),
        np.float32(L_spk.mean()),
        np.float32(L_exist.mean()),
        np.float32(L_stop.mean()),
    )
